# revision 1
# baseline (speedup 1.0000x reference)
"""DeepSet (phi -> segment_sum -> rho) Bass kernel for 8 trn2 NeuronCores.

Sharding (per hint): data-parallel over segments. 16384 segments -> 8 cores x
2048 (segment-aligned row ranges via host searchsorted on sorted segment_ids).

Per-core dataflow (all host-marshalled; T 128-row tiles per 128-seg window):
  - L1 (fp32r - 4x faster PE streaming at ~1e-5 input rounding cost):
    z1[65, 512] = W1a[8, 65].T @ xT[8, 512]; feature 64 is a constant-one
    (zero weights, bias 1) providing L2's bias via the contraction.
  - relu1 on ACT with per-partition bias -> h1a[65, 512] fp32.
  - L2 (fp32): h2[128rows, 64] = h1a_tile[65, 128].T @ W2a[65, 64] per tile -
    rows land on partitions, exactly what pooling needs; no transpose.
  - relu2 on DVE (max with 0) -> h2t[128, 256] per 4-tile group.
  - onehot[128rows, 128segs] per tile = (idr == iota), one batched DVE
    is_equal per group; padded rows have idr = -1 and match nothing.
  - pool (fp32): pooled[128segs, 64] += onehot[128, 128].T @ h2t[:, 64]
    PSUM-accumulated over the window's T tiles (matmul cost ~ out free size,
    so segments-on-partitions orientation is 2x cheaper).
  - per window: PE-transpose pooled -> [64, 128segs], copy into a [65, 512]
    4-window chunk; row 64 = per-segment counts (host bincount, DMA'd).
  - phi L3 commutes past the pooling (linear): batched tail per 512-seg
    chunk (fp32r, wide): L3 + counts*b3, then rho; out [4, 2048] per core.
Host gathers 8x[4, 2048] -> [16384, 4].
"""

import sys

import numpy as np

sys.path.insert(0, "/opt/trn_rl_repo")

import concourse.bass as bass  # noqa: E402
import concourse.mybir as mybir  # noqa: E402
import concourse.tile as tile  # noqa: E402
from concourse import bacc  # noqa: E402
from concourse.bass_utils import run_bass_kernel_spmd  # noqa: E402
from concourse.masks import make_identity  # noqa: E402

F32 = mybir.dt.float32
F32R = mybir.dt.float32r
I32 = mybir.dt.int32
AF = mybir.ActivationFunctionType

NUM_SEGMENTS = 16384
N_CORES = 8
SEG_PER_CORE = NUM_SEGMENTS // N_CORES  # 2048
WIN_SEGS = 128
N_WIN = SEG_PER_CORE // WIN_SEGS  # 16
STATE_DIM = 8
HID = 64
OUT_DIM = 4
GRP = 4  # tiles per op-batch group (512 rows)
CHUNK = 512  # segs per batched rho-tail chunk (4 windows)

import os as _os

_ABLATE = set(_os.environ.get("ABLATE", "").split(","))
_PREC = _os.environ.get("PRECISION", "l1_f32r")  # all_f32r | l1_f32r | f32
_RELU2 = _os.environ.get("RELU2", "dve")  # dve | act
_WBUFS = int(_os.environ.get("WBUFS", "4"))
_BUILD_CACHE: dict[tuple[int, int], object] = {}


def _build_program(T: int, reps: int = 1, ablate=None, precision=None):
    global _ABLATE, _PREC
    _ABLATE = set(ablate.split(",")) if ablate is not None else set(
        _os.environ.get("ABLATE", "").split(","))
    global _RELU2, _WBUFS
    _PREC = precision or _os.environ.get("PRECISION", "l1_f32r")
    _RELU2 = _os.environ.get("RELU2", "dve")
    _WBUFS = int(_os.environ.get("WBUFS", "4"))
    key = (T, reps, ",".join(sorted(_ABLATE)), _PREC, _RELU2, _WBUFS, _os.environ.get("XBUFS", "2"), _os.environ.get("H2B", "2"), _os.environ.get("TLB", "2"))
    if key in _BUILD_CACHE:
        return _BUILD_CACHE[key]
    assert T % GRP == 0
    PW = T * 128
    NG = T // GRP

    L1R = F32R if _PREC in ("all_f32r", "l1_f32r") else F32
    TLR = F32R if _PREC == "all_f32r" else F32
    nc = bacc.Bacc("TRN2", target_bir_lowering=False, debug=False, num_devices=N_CORES)

    xT_d = nc.declare_dram_parameter("xT", [STATE_DIM, N_WIN * PW], L1R, isOutput=False)
    idr_d = nc.declare_dram_parameter("idr", [128, N_WIN * T], F32, isOutput=False)
    cnt_d = nc.declare_dram_parameter("cnt", [1, SEG_PER_CORE], TLR, isOutput=False)
    w1a_d = nc.declare_dram_parameter("w1a", [STATE_DIM, HID + 1], L1R, isOutput=False)
    w2a_d = nc.declare_dram_parameter("w2a", [HID + 1, HID], F32, isOutput=False)
    w3a_d = nc.declare_dram_parameter("w3a", [HID + 1, HID], TLR, isOutput=False)
    rw1_d = nc.declare_dram_parameter("rw1", [HID, HID], TLR, isOutput=False)
    rw2_d = nc.declare_dram_parameter("rw2", [HID, HID], TLR, isOutput=False)
    rw3_d = nc.declare_dram_parameter("rw3", [HID, OUT_DIM], TLR, isOutput=False)
    pb1a_d = nc.declare_dram_parameter("pb1a", [HID + 1, 1], F32, isOutput=False)
    rb1_d = nc.declare_dram_parameter("rb1", [HID, 1], F32, isOutput=False)
    rb2_d = nc.declare_dram_parameter("rb2", [HID, 1], F32, isOutput=False)
    rb3_d = nc.declare_dram_parameter("rb3", [OUT_DIM, 1], F32, isOutput=False)
    out_d = nc.declare_dram_parameter("out", [OUT_DIM, SEG_PER_CORE], F32, isOutput=True)

    with tile.TileContext(nc) as tc:
        with (
            tc.tile_pool(name="const", bufs=1) as cpool,
            tc.tile_pool(name="xwin", bufs=int(_os.environ.get("XBUFS", "2"))) as xpool,
            tc.tile_pool(name="work", bufs=_WBUFS) as wpool,
            tc.tile_pool(name="chunk", bufs=2) as chpool,
            tc.tile_pool(name="z1ps", bufs=2, space="PSUM") as z1ps,
            tc.tile_pool(name="h2ps", bufs=int(_os.environ.get("H2B", "2")), space="PSUM") as h2ps,
            tc.tile_pool(name="poolps", bufs=2, space="PSUM") as poolps,
            tc.tile_pool(name="tailps", bufs=int(_os.environ.get("TLB", "2")), space="PSUM") as tailps,
        ):
            def cload(name, shape, dram, dt=F32):
                t = cpool.tile(shape, dt, tag=name)
                nc.sync.dma_start(out=t[:], in_=dram[:])
                return t

            w1a = cload("w1a", [STATE_DIM, HID + 1], w1a_d, L1R)
            w2a = cload("w2a", [HID + 1, HID], w2a_d)
            w3a = cload("w3a", [HID + 1, HID], w3a_d, TLR)
            rw1 = cload("rw1", [HID, HID], rw1_d, TLR)
            rw2 = cload("rw2", [HID, HID], rw2_d, TLR)
            rw3 = cload("rw3", [HID, OUT_DIM], rw3_d, TLR)
            pb1a = cload("pb1a", [HID + 1, 1], pb1a_d)
            rb1 = cload("rb1", [HID, 1], rb1_d)
            rb2 = cload("rb2", [HID, 1], rb2_d)
            rb3 = cload("rb3", [OUT_DIM, 1], rb3_d)
            idr = cload("idr", [128, N_WIN * T], idr_d)

            ident = cpool.tile([128, 128], F32, tag="ident")
            make_identity(nc, ident[:])
            iota_i = cpool.tile([128, GRP * 128], I32, tag="iota_i")
            nc.gpsimd.iota(
                iota_i[:], pattern=[[0, GRP], [1, 128]], base=0, channel_multiplier=0
            )
            iota4 = cpool.tile([128, GRP * 128], F32, tag="iota4")
            nc.vector.tensor_copy(out=iota4[:], in_=iota_i[:])

            for _rep in range(reps):
             for ch in range(SEG_PER_CORE // CHUNK):
                poolT = chpool.tile([HID + 1, CHUNK], TLR, tag="poolT")
                nc.sync.dma_start(
                    out=poolT[HID : HID + 1, :],
                    in_=cnt_d[:, ch * CHUNK : (ch + 1) * CHUNK],
                )
                for wl in range(CHUNK // WIN_SEGS):
                    w = ch * (CHUNK // WIN_SEGS) + wl
                    xw = xpool.tile([STATE_DIM, PW], L1R, tag="xw")
                    nc.sync.dma_start(out=xw[:], in_=xT_d[:, w * PW : (w + 1) * PW])

                    pooled_ps = (None if "pool" in _ABLATE
                                 else poolps.tile([WIN_SEGS, HID], F32, tag="pool"))

                    for g in range(NG):
                        gcols = slice(g * GRP * 128, (g + 1) * GRP * 128)
                        z1_ps = (None if "l1" in _ABLATE
                                 else z1ps.tile([HID + 1, GRP * 128], F32, tag="z1"))
                        if "l1" not in _ABLATE:
                            nc.tensor.matmul(
                                out=z1_ps[:], lhsT=w1a[:], rhs=xw[:, gcols],
                                start=True, stop=True,
                            )
                        if "relu1" not in _ABLATE:
                            h1a = wpool.tile([HID + 1, GRP * 128], F32, tag="h1a")
                            nc.scalar.activation(
                                out=h1a[:], in_=z1_ps[:], func=AF.Relu, bias=pb1a[:]
                            )
                        else:
                            h1a = None

                        h2_ps = (None if "l2" in _ABLATE
                                 else h2ps.tile([128, GRP * HID], F32, tag="h2"))
                        h1_src = iota4 if ("relu1" in _ABLATE or "l1" in _ABLATE) else h1a
                        for t in range(GRP) if "l2" not in _ABLATE else []:
                            nc.tensor.matmul(
                                out=h2_ps[:, t * HID : (t + 1) * HID],
                                lhsT=h1_src[:65, t * 128 : (t + 1) * 128],
                                rhs=w2a[:],
                                start=True,
                                stop=True,
                            )
                        h2t = None
                        if "relu2" not in _ABLATE:
                            h2t = wpool.tile([128, GRP * HID], F32, tag="h2t")
                            h2ps_src = iota4[:, : GRP * HID] if "l2" in _ABLATE else h2_ps[:]
                            if _RELU2 == "act":
                                nc.scalar.activation(
                                    out=h2t[:], in_=h2ps_src, func=AF.Relu, bias=0.0
                                )
                            else:
                                nc.vector.tensor_scalar(
                                    out=h2t[:], in0=h2ps_src, scalar1=0.0, scalar2=None,
                                    op0=mybir.AluOpType.max,
                                )

                        onehot = None
                        c0 = w * T + g * GRP
                        if "onehot" not in _ABLATE:
                            onehot = wpool.tile([128, GRP * 128], F32, tag="onehot")
                            nc.vector.tensor_tensor(
                                out=onehot[:].rearrange("p (a b) -> p a b", b=128),
                                in0=idr[:, c0 : c0 + GRP].to_broadcast([128, GRP, 128]),
                                in1=iota4[:].rearrange("p (a b) -> p a b", b=128),
                                op=mybir.AluOpType.is_equal,
                            )
                        oh_src = iota4 if "onehot" in _ABLATE else onehot
                        h2_src = iota4 if "relu2" in _ABLATE else h2t
                        for t in range(GRP) if "pool" not in _ABLATE else []:
                            nc.tensor.matmul(
                                out=pooled_ps[:],
                                lhsT=oh_src[:, t * 128 : (t + 1) * 128],
                                rhs=h2_src[:, t * HID : (t + 1) * HID],
                                start=(g == 0 and t == 0),
                                stop=(g == NG - 1 and t == GRP - 1),
                            )

                    if "pool" not in _ABLATE:
                        pooled_sb = wpool.tile([WIN_SEGS, HID], F32, tag="pooled")
                        nc.vector.tensor_copy(out=pooled_sb[:], in_=pooled_ps[:])
                        poolT_ps = tailps.tile([HID, WIN_SEGS], F32, tag="tail")
                        nc.tensor.transpose(
                            out=poolT_ps[:], in_=pooled_sb[:], identity=ident[:]
                        )
                        nc.vector.tensor_copy(
                            out=poolT[:HID, wl * WIN_SEGS : (wl + 1) * WIN_SEGS],
                            in_=poolT_ps[:],
                        )

                # batched phi-L3 + rho tail over this 512-seg chunk
                p3_ps = tailps.tile([HID, CHUNK], F32, tag="tail")
                nc.tensor.matmul(
                    out=p3_ps[:], lhsT=w3a[:], rhs=poolT[:], start=True, stop=True
                )
                p3 = chpool.tile([HID, CHUNK], TLR, tag="p3")
                nc.scalar.activation(out=p3[:], in_=p3_ps[:], func=AF.Copy, bias=0.0)

                r1_ps = tailps.tile([HID, CHUNK], F32, tag="tail")
                nc.tensor.matmul(
                    out=r1_ps[:], lhsT=rw1[:], rhs=p3[:], start=True, stop=True
                )
                r1 = chpool.tile([HID, CHUNK], TLR, tag="r1")
                nc.scalar.activation(out=r1[:], in_=r1_ps[:], func=AF.Relu, bias=rb1[:])

                r2_ps = tailps.tile([HID, CHUNK], F32, tag="tail")
                nc.tensor.matmul(
                    out=r2_ps[:], lhsT=rw2[:], rhs=r1[:], start=True, stop=True
                )
                r2 = chpool.tile([HID, CHUNK], TLR, tag="r2")
                nc.scalar.activation(out=r2[:], in_=r2_ps[:], func=AF.Relu, bias=rb2[:])

                r3_ps = tailps.tile([OUT_DIM, CHUNK], F32, tag="tail")
                nc.tensor.matmul(
                    out=r3_ps[:], lhsT=rw3[:], rhs=r2[:], start=True, stop=True
                )
                out_sb = chpool.tile([OUT_DIM, CHUNK], F32, tag="outc")
                nc.vector.tensor_scalar(
                    out=out_sb[:], in0=r3_ps[:], scalar1=rb3[:], scalar2=None,
                    op0=mybir.AluOpType.add,
                )
                nc.sync.dma_start(
                    out=out_d[:, ch * CHUNK : (ch + 1) * CHUNK], in_=out_sb[:]
                )

    nc.compile()
    _BUILD_CACHE[key] = nc
    return nc


def _prep_inputs(neighbors: np.ndarray, segment_ids: np.ndarray):
    """Shard rows by 128-segment windows; pad each window to T 128-row tiles."""
    x = np.asarray(neighbors, dtype=np.float32)
    ids = np.asarray(segment_ids, dtype=np.int64)
    n_gwin = NUM_SEGMENTS // WIN_SEGS
    edges = np.searchsorted(ids, np.arange(0, NUM_SEGMENTS + 1, WIN_SEGS))
    wcnt = np.diff(edges)
    T = max(GRP, GRP * int(np.ceil(wcnt.max() / (128 * GRP))))
    PW = T * 128

    xT = np.zeros((N_CORES, STATE_DIM, N_WIN * PW), dtype=np.float32)
    idr = np.full((N_CORES, 128, N_WIN * T), -1.0, dtype=np.float32)
    counts = np.bincount(ids, minlength=NUM_SEGMENTS).astype(np.float32)
    cnt = counts.reshape(N_CORES, 1, SEG_PER_CORE)
    for g in range(n_gwin):
        c, wl = divmod(g, N_WIN)
        r0, r1 = int(edges[g]), int(edges[g + 1])
        n = r1 - r0
        if n == 0:
            continue
        base = wl * PW
        xT[c, :, base : base + n] = x[r0:r1].T
        rel = np.full(PW, -1.0, dtype=np.float32)
        rel[:n] = (ids[r0:r1] - g * WIN_SEGS).astype(np.float32)
        idr[c, :, wl * T : (wl + 1) * T] = rel.reshape(T, 128).T
    return xT, idr, cnt, T


def prep_maps(inputs: dict):
    """Host-side marshalling: returns (T, in_maps per core)."""
    xT, idr, cnt, T = _prep_inputs(inputs["neighbors"], inputs["segment_ids"])
    f = lambda a: np.ascontiguousarray(np.asarray(a, dtype=np.float32))
    col = lambda a: f(a).reshape(-1, 1)
    w1a = np.concatenate([f(inputs["phi_W1"]), np.zeros((STATE_DIM, 1), np.float32)], 1)
    pb1a = np.concatenate([col(inputs["phi_b1"]), np.ones((1, 1), np.float32)], 0)
    w2a = np.vstack([f(inputs["phi_W2"]), f(inputs["phi_b2"]).reshape(1, -1)])
    w3a = np.vstack([f(inputs["phi_W3"]), f(inputs["phi_b3"]).reshape(1, -1)])
    shared = {
        "w1a": w1a,
        "w2a": w2a,
        "w3a": w3a,
        "rw1": f(inputs["rho_W1"]),
        "rw2": f(inputs["rho_W2"]),
        "rw3": f(inputs["rho_W3"]),
        "pb1a": pb1a,
        "rb1": col(inputs["rho_b1"]),
        "rb2": col(inputs["rho_b2"]),
        "rb3": col(inputs["rho_b3"]),
    }
    in_maps = [
        {"xT": xT[c], "idr": idr[c], "cnt": cnt[c], **shared} for c in range(N_CORES)
    ]
    return T, in_maps


def kernel(**inputs):
    T, in_maps = prep_maps(inputs)
    nc = _build_program(T)
    res = run_bass_kernel_spmd(nc, in_maps, core_ids=list(range(N_CORES)))
    out = np.concatenate(
        [res.results[c]["out"].T for c in range(N_CORES)], axis=0
    ).astype(np.float32)
    return out



# revision 28
# speedup vs baseline: 29.1240x; 29.1240x over previous
"""DeepSet (phi -> segment_sum -> rho) Bass kernel for 8 trn2 NeuronCores.

Sharding (per hint): data-parallel over segments. 16384 segments -> 8 cores x
2048 (segment-aligned row ranges via host searchsorted on sorted segment_ids).

v2 dataflow (per core; flat stream of 128-row tiles, supergroups of 8 tiles):
  - L1 (fp32r, F=512 so 1 cyc/row): two matmuls pack 2x64 features onto the
    128 PSUM partitions -> z1 [128, 512] covers 8 tiles (1024 rows).
  - relu1: ONE ACT op per supergroup [128, 512] fp32->fp16 with stacked
    per-partition bias. Halves ACT vs the 65-row layout.
  - L2 (fp16, 1 cyc/row): bias first via ones-lhsT matmul (rhs = b2 tiled),
    then 8 matmuls h1[64, 128].T @ w2[64, 64] accumulate -> h2 [128rows, 512].
  - relu2 (max 0) fp32->fp16, alternating DVE / ACT by supergroup for balance.
  - onehot [rows, (seg, tile)] fp16 via is_equal in (b a) layout: every
    operand keeps a packed 2-byte last dim -> DVE 2x_1p fast mode.
    Alternating DVE / gpsimd by supergroup for balance.
  - pool (fp16): pooled[128segs, 64] += onehot[:, t::8].T @ h2t[:, 64t:]
    PSUM-accumulated over each 128-seg window's T tiles.
  - per window: PE-transpose pooled -> [64, 128segs]; row 64 = counts.
  - tail (fp32r, F=512 so 1 cyc/row): phi-L3 (commutes past pooling) + rho
    per 512-seg chunk; out [4, 2048] per core.
Host gathers 8x[4, 2048] -> [16384, 4].
"""

import os as _os
import sys

import numpy as np

sys.path.insert(0, "/opt/trn_rl_repo")

import concourse.bass as bass  # noqa: E402
import concourse.mybir as mybir  # noqa: E402
import concourse.tile as tile  # noqa: E402
from concourse import bacc  # noqa: E402
from concourse.bass_utils import run_bass_kernel_spmd  # noqa: E402
from concourse.masks import make_identity  # noqa: E402

F32 = mybir.dt.float32
F32R = mybir.dt.float32r
F16 = mybir.dt.float16
I32 = mybir.dt.int32
AF = mybir.ActivationFunctionType
AOP = mybir.AluOpType

NUM_SEGMENTS = 16384
N_CORES = 8
SEG_PER_CORE = NUM_SEGMENTS // N_CORES  # 2048
WIN_SEGS = 128
NWV = 17  # variable-boundary windows per core (<=128 segs, <=T*128 rows)
STATE_DIM = 8
HID = 64
OUT_DIM = 4
SG = 8  # tiles per supergroup (2 packed halves of 4)
CHUNK = 512  # segs per batched rho-tail chunk (4 windows)
OUTW = NWV * WIN_SEGS  # padded output columns per core (2176)
B16C = 1028 + OUTW  # fp16 const-blob columns

_BUILD_CACHE: dict = {}


def _build_program(T: int, reps: int = 1, ablate=None, precision=None):
    ablate = set((ablate if ablate is not None
                  else _os.environ.get("ABLATE", "")).split(","))
    r2act = int(_os.environ.get("R2ACT", "3"))  # relu2 on ACT when sg%k==0
    ohpool = int(_os.environ.get("OHPOOL", "0"))  # onehot on gpsimd when sg%k==0 (0=never; TT is not a valid Pool opcode)
    xbufs = int(_os.environ.get("XBUFS", "3"))
    wbufs = int(_os.environ.get("WBUFS", "4"))
    key = (T, reps, ",".join(sorted(ablate)), r2act, ohpool, xbufs, wbufs,
           _os.environ.get("XSG", "12"))
    if key in _BUILD_CACHE:
        return _BUILD_CACHE[key]
    assert T % 4 == 0
    PW = T * 128  # rows per window
    NT = NWV * T  # tiles per core
    assert NT % SG == 0
    NSG = NT // SG

    nc = bacc.Bacc("TRN2", target_bir_lowering=False, debug=False, num_devices=N_CORES)

    xT_d = nc.declare_dram_parameter("xT", [2 * STATE_DIM, NSG * 512], F16, isOutput=False)
    idr_d = nc.declare_dram_parameter("idr", [128, NT], F16, isOutput=False)
    b16_d = nc.declare_dram_parameter("b16", [128, B16C], F16, isOutput=False)
    b32_d = nc.declare_dram_parameter("b32", [128, 4], F32, isOutput=False)
    out_d = nc.declare_dram_parameter("out", [OUT_DIM, OUTW], F32, isOutput=True)

    with tile.TileContext(nc) as tc:
        with (
            tc.tile_pool(name="const", bufs=1) as cpool,
            tc.tile_pool(name="xwin", bufs=xbufs) as xpool,
            tc.tile_pool(name="work", bufs=wbufs) as wpool,
            tc.tile_pool(name="chunk", bufs=2) as chpool,
            tc.tile_pool(name="z1ps", bufs=2, space="PSUM") as z1ps,
            tc.tile_pool(name="h2ps", bufs=int(_os.environ.get("H2B", "1")), space="PSUM") as h2ps,
            tc.tile_pool(name="poolps", bufs=2, space="PSUM") as poolps,
            tc.tile_pool(name="tailps", bufs=int(_os.environ.get("TLB", "1")), space="PSUM") as tailps,
        ):
            def cload(name, shape, dram, dt=F32):
                t = cpool.tile(shape, dt, tag=name)
                nc.sync.dma_start(out=t[:], in_=dram[:])
                return t

            b16 = cload("b16", [128, B16C], b16_d, F16)
            b32 = cload("b32", [128, 4], b32_d)
            idr = cload("idr", [128, NT], idr_d, F16)
            w2a = b16[:, 0:128]
            w3i = b16[:, 128:192]
            rw1 = b16[0:HID, 192:256]
            rw2 = b16[0:HID, 256:320]
            rw3 = b16[0:HID, 320:324]
            w1a = b16[0 : 2 * STATE_DIM, 324:452]
            b3r = b16[0:1, 452:516]
            b2rep = b16[0:1, 516 : 516 + SG * HID]
            cnt1 = b16[0:1, 1028 : 1028 + OUTW]
            pb1p = b32[:, 0:1]
            rb1 = b32[0:HID, 1:2]
            rb2 = b32[0:HID, 2:3]
            rb3 = b32[0:OUT_DIM, 3:4]

            ones1 = cpool.tile([1, 128], F16, tag="ones1")
            nc.gpsimd.memset(ones1[:], 1.0)
            iota_i = cpool.tile([128, 128 * SG], I32, tag="iota_i")
            nc.gpsimd.iota(
                iota_i[:], pattern=[[1, 128], [0, SG]], base=0,
                channel_multiplier=0,
            )
            iota_ba = cpool.tile([128, 128 * SG], F16, tag="iota_ba")
            nc.vector.tensor_copy(out=iota_ba[:], in_=iota_i[:])
            # ablation fallbacks
            c_h1 = cpool.tile([128, SG * 64], F16, tag="c_h1")
            nc.gpsimd.memset(c_h1[:], 1.0)

            def _tail_steps(ch, poolT, W):
                st = {}

                def s0():
                    st["p3_ps"] = tailps.tile([HID, CHUNK], F32, tag="tail", name="p3_ps")
                    nc.tensor.matmul(
                        out=st["p3_ps"][:, :W], lhsT=w3i, rhs=poolT[:, :W],
                        start=True, stop=False,
                    )
                    nc.tensor.matmul(
                        out=st["p3_ps"][:, :W], lhsT=b3r,
                        rhs=cnt1[:, ch * CHUNK : ch * CHUNK + W],
                        start=False, stop=True,
                    )

                def s1():
                    st["p3"] = chpool.tile([HID, CHUNK], F16, tag="p3", name="p3")
                    nc.scalar.activation(
                        out=st["p3"][:, :W], in_=st["p3_ps"][:, :W], func=AF.Copy, bias=0.0
                    )

                def s2():
                    st["r1_ps"] = tailps.tile([HID, CHUNK], F32, tag="tail", name="r1_ps")
                    nc.tensor.matmul(
                        out=st["r1_ps"][:, :W], lhsT=rw1, rhs=st["p3"][:, :W],
                        start=True, stop=True,
                    )

                def s3():
                    st["r1"] = chpool.tile([HID, CHUNK], F16, tag="r1", name="r1")
                    nc.scalar.activation(
                        out=st["r1"][:, :W], in_=st["r1_ps"][:, :W], func=AF.Relu,
                        bias=rb1,
                    )

                def s4():
                    st["r2_ps"] = tailps.tile([HID, CHUNK], F32, tag="tail", name="r2_ps")
                    nc.tensor.matmul(
                        out=st["r2_ps"][:, :W], lhsT=rw2, rhs=st["r1"][:, :W],
                        start=True, stop=True,
                    )

                def s5():
                    st["r2"] = chpool.tile([HID, CHUNK], F16, tag="r2", name="r2")
                    nc.scalar.activation(
                        out=st["r2"][:, :W], in_=st["r2_ps"][:, :W], func=AF.Relu,
                        bias=rb2,
                    )

                def s6():
                    st["r3_ps"] = tailps.tile([OUT_DIM, CHUNK], F32, tag="tail", name="r3_ps")
                    nc.tensor.matmul(
                        out=st["r3_ps"][:, :W], lhsT=rw3, rhs=st["r2"][:, :W],
                        start=True, stop=True,
                    )

                def s7():
                    st["out_sb"] = chpool.tile([OUT_DIM, CHUNK], F32, tag="outc", name="out_sb")
                    nc.scalar.activation(
                        out=st["out_sb"][:, :W], in_=st["r3_ps"][:, :W], func=AF.Identity,
                        bias=rb3,
                    )
                    nc.sync.dma_start(
                        out=out_d[:, ch * CHUNK : ch * CHUNK + W],
                        in_=st["out_sb"][:, :W],
                    )

                return [s0, s1, s2, s3, s4, s5, s6, s7]

            for _rep in range(reps):
                xc = None
                poolT = None
                pooled_ps = None
                pending = []
                XSG = int(_os.environ.get("XSG", "12"))  # supergroups per x-chunk DMA
                z1_ps = None
                h1pack2 = None
                h2_ps2 = None
                h2t2 = None
                for sg in range(NSG):
                    j0 = sg * SG
                    par = sg % 2
                    if pending:
                        pending.pop(0)()
                    if sg % XSG == 0:
                        xc = xpool.tile([2 * STATE_DIM, XSG * 512], F16, tag="xc")
                        wcols = min(XSG, NSG - sg) * 512
                        nc.sync.dma_start(
                            out=xc[:, :wcols],
                            in_=xT_d[:, sg * 512 : sg * 512 + wcols],
                        )
                    # ---- K-packed L1, pair-batched z1 [128, 1024] ----
                    if par == 0 and "l1" not in ablate:
                        z1_ps = z1ps.tile([128, 1024], F32, tag="z1")
                    if "l1" not in ablate:
                        nc.tensor.matmul(
                            out=z1_ps[:, par * 512 : par * 512 + 512],
                            lhsT=w1a,
                            rhs=xc[:, (sg % XSG) * 512 : (sg % XSG) * 512 + 512],
                            start=True, stop=True,
                        )
                    # ---- relu1 (ACT) once per pair -> h1pack2 fp16 [128, 1024] ----
                    if "relu1" not in ablate and "l1" not in ablate:
                        if par == 1:
                            h1pack2 = wpool.tile([128, 1024], F16, tag="h1pack")
                            nc.scalar.activation(
                                out=h1pack2[:], in_=z1_ps[:], func=AF.Relu,
                                bias=pb1p,
                            )
                        if par == 0:
                            continue_l2 = False  # L2 for the pair runs on par==1
                    else:
                        h1pack2 = None
                    if par == 0:
                        continue
                    # ---- L2 + bias for BOTH sgs of the pair -> h2_ps2 [128, 1024] ----
                    h1src = h1pack2 if h1pack2 is not None else None
                    h2_ps2 = (None if "l2" in ablate
                              else h2ps.tile([128, 2 * SG * HID], F32, tag="h2"))
                    if "l2" not in ablate:
                        for half in range(2):
                            nc.tensor.matmul(
                                out=h2_ps2[:, half * 512 : half * 512 + 512],
                                lhsT=ones1[:], rhs=b2rep,
                                start=True, stop=False,
                            )
                        for tl8 in range(8):
                            half, tl = divmod(tl8, 4)
                            src_ = (h1src[:, half * 512 + tl * 128 : half * 512 + (tl + 1) * 128]
                                    if h1src is not None else c_h1[:, 0:128])
                            nc.tensor.matmul(
                                out=h2_ps2[:, half * 512 + tl * 128 : half * 512 + (tl + 1) * 128],
                                lhsT=src_,
                                rhs=w2a,
                                start=False, stop=True,
                            )
                    # ---- relu2 once per pair -> h2t2 fp16 [128, 1024] ----
                    if "relu2" not in ablate and "l2" not in ablate:
                        h2t2 = wpool.tile([128, 2 * SG * HID], F16, tag="h2t")
                        if r2act and (sg // 2) % r2act == 0:
                            nc.scalar.activation(
                                out=h2t2[:], in_=h2_ps2[:], func=AF.Relu, bias=0.0
                            )
                        else:
                            nc.vector.tensor_scalar(
                                out=h2t2[:], in0=h2_ps2[:], scalar1=0.0,
                                scalar2=None, op0=AOP.max,
                            )
                    else:
                        h2t2 = None
                    # ---- onehot for both sgs of the pair ----
                    if "onehot" not in ablate:
                        onehot = wpool.tile([128, 128 * 2 * SG], F16, tag="onehot")
                        nc.vector.tensor_tensor(
                            out=onehot[:].rearrange("p (b a) -> p b a", a=2 * SG),
                            in0=idr[:, j0 - SG : j0 + SG].unsqueeze(1)
                                .broadcast_to([128, 128, 2 * SG]),
                            in1=iota_ba[:].rearrange("p (b a) -> p b a", a=2 * SG),
                            op=AOP.is_equal,
                        )
                    else:
                        onehot = iota_ba
                    # ---- pool matmuls + window/chunk tails (both sgs) ----
                    for t in range(2 * SG) if "pool" not in ablate else []:
                        j = j0 - SG + t
                        w, tloc = divmod(j, T)
                        if tloc == 0:
                            pooled_ps = poolps.tile([WIN_SEGS, HID], F32, tag="pool")
                            if w % 4 == 0:
                                poolT = chpool.tile([2 * HID, CHUNK], F16, tag="poolT")
                        half_, t_ = divmod(t, SG)
                        g_, tl_ = divmod(t_, 4)
                        hcol = half_ * 512 + tl_ * 128 + g_ * HID
                        nc.tensor.matmul(
                            out=pooled_ps[:],
                            lhsT=onehot[:, t : 128 * 2 * SG : 2 * SG],
                            rhs=h2t2[:, hcol : hcol + HID],
                            start=(tloc == 0), stop=(tloc == T - 1),
                        )
                        if tloc == T - 1:
                            wl = w % 4

                            stw = {}

                            def wc0(pp=pooled_ps, st_=stw):
                                st_["p16"] = wpool.tile(
                                    [WIN_SEGS, 2 * HID], F16, tag="pooled",
                                    name="pooled16",
                                )
                                nc.scalar.activation(
                                    out=st_["p16"][:].rearrange(
                                        "p (c i) -> p c i", i=2
                                    ),
                                    in_=pp[:].unsqueeze(2)
                                        .broadcast_to([WIN_SEGS, HID, 2]),
                                    func=AF.Copy, bias=0.0,
                                )

                            def wc1(pt=poolT, wl_=wl, st_=stw):
                                nc.sync.dma_start_transpose(
                                    out=pt[:, wl_ * WIN_SEGS : (wl_ + 1) * WIN_SEGS],
                                    in_=st_["p16"][:],
                                )

                            pending.append(wc0)
                            pending.append(wc1)
                            if wl == 3:
                                pending.extend(_tail_steps(w // 4, poolT))
                for th in pending:
                    th()

    nc.compile()
    _BUILD_CACHE[key] = nc
    return nc


def _prep_inputs(neighbors: np.ndarray, segment_ids: np.ndarray):
    """Equal-row variable windows: NWV windows/core, each <=128 segs and
    <=T*128 rows (greedy), padded to T 128-row tiles."""
    x = np.asarray(neighbors, dtype=np.float32)
    ids = np.asarray(segment_ids, dtype=np.int64)
    counts_g = np.bincount(ids, minlength=NUM_SEGMENTS)
    seg_edges = np.searchsorted(ids, np.arange(NUM_SEGMENTS + 1))

    def partition(core, T):
        cap = T * 128
        cnts = counts_g[core * SEG_PER_CORE : (core + 1) * SEG_PER_CORE]
        wins = []
        s = 0
        while s < SEG_PER_CORE:
            e = min(s + WIN_SEGS, SEG_PER_CORE)
            rows = 0
            e2 = s
            while e2 < e and rows + cnts[e2] <= cap:
                rows += cnts[e2]
                e2 += 1
            if e2 == s:
                return None
            wins.append((s, e2))
            s = e2
        if len(wins) > NWV:
            return None
        while len(wins) < NWV:
            wins.append((SEG_PER_CORE, SEG_PER_CORE))
        return wins

    T = 28
    while True:
        allw = [partition(c, T) for c in range(N_CORES)]
        if all(w is not None for w in allw):
            break
        T += 4
    PW = T * 128
    NT = NWV * T

    xT = np.zeros((N_CORES, STATE_DIM, NT * 128), dtype=np.float16)
    idr = np.full((N_CORES, 128, NT), -1.0, dtype=np.float16)
    cntp = np.zeros((N_CORES, 1, OUTW), dtype=np.float16)
    for c in range(N_CORES):
        base_seg = c * SEG_PER_CORE
        for w, (s0, s1) in enumerate(allw[c]):
            n_segs = s1 - s0
            if n_segs == 0:
                continue
            r0 = int(seg_edges[base_seg + s0])
            r1 = int(seg_edges[base_seg + s1])
            n = r1 - r0
            cntp[c, 0, w * WIN_SEGS : w * WIN_SEGS + n_segs] = counts_g[
                base_seg + s0 : base_seg + s1
            ].astype(np.float16)
            if n == 0:
                continue
            base = w * PW
            xT[c, :, base : base + n] = x[r0:r1].T
            rel = np.full(PW, -1.0, dtype=np.float32)
            rel[:n] = (ids[r0:r1] - (base_seg + s0)).astype(np.float32)
            idr[c, :, w * T : (w + 1) * T] = rel.reshape(T, 128).T.astype(np.float16)
    # K-packed per-supergroup layout: rows 0-7 = half A (4 tiles), 8-15 = half B
    NSG = NT // SG
    xT2 = np.zeros((N_CORES, 2 * STATE_DIM, NSG * 512), dtype=np.float16)
    for s in range(NSG):
        c0 = s * SG * 128
        xT2[:, :STATE_DIM, s * 512 : (s + 1) * 512] = xT[:, :, c0 : c0 + 512]
        xT2[:, STATE_DIM:, s * 512 : (s + 1) * 512] = xT[:, :, c0 + 512 : c0 + 1024]
    return xT2, idr, cntp, T, allw


def _w1blk(w1: np.ndarray) -> np.ndarray:
    blk = np.zeros((2 * STATE_DIM, 128), dtype=np.float16)
    blk[:STATE_DIM, :HID] = w1.astype(np.float16)
    blk[STATE_DIM:, HID:] = w1.astype(np.float16)
    return blk


def _w2blk(w2: np.ndarray) -> np.ndarray:
    out = np.zeros((2 * HID, 2 * HID), dtype=np.float16)
    out[:HID, :HID] = w2.astype(np.float16)
    out[HID:, HID:] = w2.astype(np.float16)
    return out


def _w3int(w3: np.ndarray) -> np.ndarray:
    out = np.zeros((2 * HID, HID), dtype=np.float16)
    out[0::2] = w3.astype(np.float16)
    return out


def prep_maps(inputs: dict):
    """Host-side marshalling: returns (T, in_maps per core)."""
    xT, idr, cnt, T, allw = _prep_inputs(inputs["neighbors"], inputs["segment_ids"])
    prep_maps.windows = allw
    f = lambda a: np.ascontiguousarray(np.asarray(a, dtype=np.float32))
    col = lambda a: f(a).reshape(-1, 1)
    b1 = col(inputs["phi_b1"])
    b16 = np.zeros((N_CORES, 128, B16C), dtype=np.float16)
    b16[:, :, 0:128] = _w2blk(f(inputs["phi_W2"]))
    b16[:, :, 128:192] = _w3int(f(inputs["phi_W3"]))
    b16[:, 0:HID, 192:256] = f(inputs["rho_W1"]).astype(np.float16)
    b16[:, 0:HID, 256:320] = f(inputs["rho_W2"]).astype(np.float16)
    b16[:, 0:HID, 320:324] = f(inputs["rho_W3"]).astype(np.float16)
    b16[:, 0 : 2 * STATE_DIM, 324:452] = _w1blk(f(inputs["phi_W1"]))
    b16[:, 0, 452:516] = f(inputs["phi_b3"]).astype(np.float16)
    b16[:, 0, 516 : 516 + SG * HID] = np.tile(
        f(inputs["phi_b2"]), SG
    ).astype(np.float16)
    b16[:, 0, 1028 : 1028 + OUTW] = cnt[:, 0, :].astype(np.float16)
    b32 = np.zeros((N_CORES, 128, 4), dtype=np.float32)
    b32[:, :, 0] = np.concatenate([b1, b1], axis=0)[:, 0]
    b32[:, 0:HID, 1] = col(inputs["rho_b1"])[:, 0]
    b32[:, 0:HID, 2] = col(inputs["rho_b2"])[:, 0]
    b32[:, 0:OUT_DIM, 3] = col(inputs["rho_b3"])[:, 0]
    in_maps = [
        {"xT": xT[c], "idr": idr[c], "b16": b16[c], "b32": b32[c]}
        for c in range(N_CORES)
    ]
    return T, in_maps


def kernel(**inputs):
    T, in_maps = prep_maps(inputs)
    allw = prep_maps.windows
    nc = _build_program(T)
    res = run_bass_kernel_spmd(nc, in_maps, core_ids=list(range(N_CORES)))
    out = np.zeros((NUM_SEGMENTS, OUT_DIM), dtype=np.float32)
    for c in range(N_CORES):
        oc = res.results[c]["out"]  # [4, OUTW]
        for w, (s0, s1) in enumerate(allw[c]):
            if s1 > s0:
                out[c * SEG_PER_CORE + s0 : c * SEG_PER_CORE + s1] = oc[
                    :, w * WIN_SEGS : w * WIN_SEGS + (s1 - s0)
                ].T
    return out
